# revision 106
# baseline (speedup 1.0000x reference)
"""Tensor-parallel (over GQA head groups) multi-head attention for 8 trn2 cores.

Each core owns 4 query heads + their shared kv head (one GQA group), the
matching 384 rows of wqkv and 256 columns of wo.  Every core computes a full
[S, D] bf16 partial of the output projection; the host sums the 8 partials.

Dataflow inside one core (bf16 data plane, fp32 PSUM):
  qkv projection in fp8-e4m3 hi/lo split form via DoubleRow matmuls
    (h = ha+hb, w*256 = wa+wb host-split; qkv = ha@wa + hb@wa + ha@wb,
     each term packing two 128-deep k-tiles per PE instruction at
     0.5 cyc/row -> 0.75x the bf16 column count).  The hf=0 half runs all
     three m-chunks in 6 transient PSUM banks, streaming g-pair chunks as
     their (big, few) DMAs land; the hf=1 half is pumped into the early
     attention j-loops using the persistent 2-bank acc pool.
  rope on DVE via plain tensor_mul/add (2x bf16 fast path -- a
    scalar_tensor_tensor gain operand would disable it), with dedicated
    q tables (x 1/(8*256)) and k tables (x 1/256) so no gain scalar is
    needed; critical hf=0 chains run in 512-col chunks.
  scores per (qs-512-chunk c, head-pair hp, ks-128-block j) in bf16 with
    causally-dead columns trimmed from the moving operand; exp on ScalarE
    straight from PSUM into a bf16 ex tile (deep ex pool so PV never waits
    on rotation); triangle mask multiply on DVE; PV accumulates [65, live] (ones column = softmax
    denominator row) into a single [128,1024] PSUM slot with lazy-zero
    multi-writer accumulation.
  softmax normalization: po -> poc (bf16 sbuf, DVE), then
    per 512-half: ones-row matmul broadcasts the denominator back into po's
    dead rows, one reciprocal evacuates it to sbuf, and GPSIMD does the o2
    multiply; odd head moved across partitions by a small sbuf-sbuf DMA
    (odd half first -- out-projection waits on it).
  out-projection per (c, seq-128-block b): 4x[128,512] psum tiles from
    o2a/o2b against bf16 wo, copied to a [128,2048] staging tile (DVE
    mostly) and DMA'd to DRAM as bf16.  For c=0..2 the work is emitted as
    16 FINE-grained one-psum-tile units per c, queued after each c's norm
    completes and pumped into the following attention j-loops at rates
    tuned per c (half-rate for c=2,3 so the units spread across both
    head-pair sweeps instead of clumping); c=3 runs as a dedicated tail:
    the attention PSUM pool is closed after the final norm and a fresh
    6-bank pool lets 6 o2a-half matmuls pre-start during the norm chain,
    with the 16 pp tiles software-pipelined (depth 6) and per-1024-column
    output DMAs.
  All DMAs are big and few (~650ns serialized issue each), and the h
  g-pair stream is kept contiguous: weight/table loads are either lagged
  behind the m-chunks that need them (wq m0/m1 at t=1/2) or deferred
  until after the last h byte (rope tables, triangle, wo) so the first
  attention chunk starts as early as possible.  Outputs are bf16, summed
  in fp32 on the host.  An fp8-split out-projection path exists behind
  FP8C but measures slower in the timeline model.
"""

import sys

if "/opt/trn_rl_repo" not in sys.path:
    sys.path.insert(0, "/opt/trn_rl_repo")

import numpy as np

S = 2048
D = 2048
HD = 64
N_HEAD = 32
N_KV = 8
NCORES = 8
KV_SIZE = N_KV * HD  # 512
WSCALE = 256.0
FP8C = -1  # out-projection uses fp8-split DR for c <= FP8C (-1 disables)

_CACHE = {}


def _build_module():
    from contextlib import ExitStack

    import concourse.mybir as mybir
    import concourse.tile as tile
    from concourse import bacc
    from concourse.bass import ds

    FP = mybir.dt.float32
    BF = mybir.dt.bfloat16
    F8 = mybir.dt.float8e4
    EXP = mybir.ActivationFunctionType.Exp
    DR = mybir.MatmulPerfMode.DoubleRow
    MUL = mybir.AluOpType.mult

    nc = bacc.Bacc(
        "TRN2",
        target_bir_lowering=False,
        debug=False,
        enable_asserts=False,
        num_devices=NCORES,
    )

    # [p, 2048*g + s] = split(h[s, 128*g + p])
    h8a = nc.dram_tensor("h8a", [128, 16 * S], F8, kind="ExternalInput").ap()
    h8b = nc.dram_tensor("h8b", [128, 16 * S], F8, kind="ExternalInput").ap()
    # [p, 2048*m + 128*g + r] = split(256 * wl[128*m + r, 128*g + p]);
    # m: 0 q-heads 0/1, 1 q-heads 2/3, 2 k+v
    wqa = nc.dram_tensor("wqa", [128, 16 * 384], F8, kind="ExternalInput").ap()
    wqb = nc.dram_tensor("wqb", [128, 16 * 384], F8, kind="ExternalInput").ap()
    # [p, 2048*u + e] = wo[e, 256*core + 128*u + p] / 256
    wo = nc.dram_tensor("wo", [128, 2 * 2048], BF, kind="ExternalInput").ap()
    # fp8 hi/lo split of 256 * wo[e, 256*core + 128*u + p] (x65536 vs wo16;
    # folded out on the host for the c<=2 seq rows)
    wo8a = nc.dram_tensor("wo8a", [128, 2 * 2048], F8, kind="ExternalInput").ap()
    wo8b = nc.dram_tensor("wo8b", [128, 2 * 2048], F8, kind="ExternalInput").ap()
    # q tables: [p, s] cos (cols 0:2048) | sin' (2048:4096), /(8*256);
    # k tables: same at cols 4096:8192 but /256 (dedicated tables instead of a
    # gain scalar: an fp32 scalar operand disables the DVE 2x/4x fast paths)
    rq = nc.dram_tensor("rq", [128, 4 * S], BF, kind="ExternalInput").ap()
    # cols 0:128 tri[p, f] = (p <= f); cols 128:256 identity[p, f] = (p == f)
    tri = nc.dram_tensor("tri", [128, 256], BF, kind="ExternalInput").ap()
    out = nc.dram_tensor("out", [S, D], BF, kind="ExternalOutput").ap()

    with tile.TileContext(nc) as tc, ExitStack() as ctx, nc.allow_low_precision(
        reason="bf16 data plane by design; fp32 psum accumulation throughout"
    ):
        const = ctx.enter_context(tc.tile_pool(name="const", bufs=1))
        wqa_sb = const.tile([128, 3, 16, 128], F8, tag="wqa")
        wqb_sb = const.tile([128, 3, 16, 128], F8, tag="wqb")
        wo_sb = const.tile([128, 4096], BF, tag="wo")
        wo8a_sb = const.tile([128, 2, 2048], F8, tag="wo8a")
        wo8b_sb = const.tile([128, 2, 2048], F8, tag="wo8b")
        rq_sb = const.tile([128, 8192], BF, tag="rq")
        tri_sb = const.tile([128, 256], BF, tag="tri")
        ident64 = tri_sb[64:128, 192:256]
        ones_row = tri_sb[64:65, 64:128]

        hpool = ctx.enter_context(tc.tile_pool(name="hp", bufs=1))
        ha_sb = hpool.tile([128, 16, 2048], F8, tag="ha")
        hb_sb = hpool.tile([128, 16, 2048], F8, tag="hb")

        rawp = ctx.enter_context(tc.tile_pool(name="rawp", bufs=1))
        q01r = rawp.tile([128, S], BF, tag="q01r")
        q23r = rawp.tile([128, S], BF, tag="q23r")
        kvr = rawp.tile([128, S], BF, tag="kvr")
        rotp = ctx.enter_context(tc.tile_pool(name="rotp", bufs=1))
        q01s = rotp.tile([128, S], BF, tag="q01s")
        q23s = rotp.tile([128, S], BF, tag="q23s")
        ks = rotp.tile([128, S], BF, tag="ks")
        vpool = ctx.enter_context(tc.tile_pool(name="vsb", bufs=1))
        v65 = vpool.tile([128, 16, 65], BF, tag="v")
        scr = ctx.enter_context(tc.tile_pool(name="scr", bufs=2))
        expool = ctx.enter_context(tc.tile_pool(name="ex", bufs=6))
        pocp = ctx.enter_context(tc.tile_pool(name="pocp", bufs=3))
        rbrp = ctx.enter_context(tc.tile_pool(name="rbrp", bufs=3))
        o2p = ctx.enter_context(tc.tile_pool(name="o2p", bufs=1))
        o2a = o2p.tile([128, S], BF, tag="o2a")
        o2b = o2p.tile([128, S], BF, tag="o2b")
        o28h = o2p.tile([128, 2, S], F8, tag="o28h")
        o28l = o2p.tile([128, 2, S], F8, tag="o28l")
        nmp = ctx.enter_context(tc.tile_pool(name="nmp", bufs=2))
        stp = ctx.enter_context(tc.tile_pool(name="stp", bufs=5))

        # PSUM: persistent acc pool (2 banks) + transient hf0 projection pool
        # (6 banks, closed before attention) + attention pool (sc 4 + po 2)
        psA = ctx.enter_context(tc.tile_pool(name="psA", bufs=1, space="PSUM"))
        psA.tile([128, 512], FP, tag="acc", bufs=2, name="acc_reserve")
        p6ctx = ExitStack()
        psP6 = p6ctx.enter_context(tc.tile_pool(name="psP6", bufs=1, space="PSUM"))
        P = {}

        # ---------------- DMA staging ----------------
        # DMA issue overhead (~650ns serialized per DMA) dominates over
        # transfer for small DMAs, so use few big ones: k+v weight block
        # first, then g-PAIR h chunks (matching DoubleRow consumption order).
        def wq_dma(m):
            nc.sync.dma_start(
                wqa_sb[:, m].rearrange("p a b -> p (a b)"), wqa[:, ds(2048 * m, 2048)]
            )
            nc.sync.dma_start(
                wqb_sb[:, m].rearrange("p a b -> p (a b)"), wqb[:, ds(2048 * m, 2048)]
            )

        wq_dma(2)
        for t in range(8):
            nc.sync.dma_start(
                ha_sb[:, 2 * t : 2 * t + 2, :].rearrange("p a b -> p (a b)"),
                h8a[:, ds(4096 * t, 4096)],
            )
            nc.sync.dma_start(
                hb_sb[:, 2 * t : 2 * t + 2, :].rearrange("p a b -> p (a b)"),
                h8b[:, ds(4096 * t, 4096)],
            )
            if t == 1:
                wq_dma(0)
            if t == 2:
                wq_dma(1)
        # hf=0 table halves gate the first ropes (k first); the rest defers
        nc.sync.dma_start(rq_sb[0:64, 4096:5120], rq[0:64, ds(4096, 1024)])
        nc.sync.dma_start(rq_sb[0:64, 6144:7168], rq[0:64, ds(6144, 1024)])
        nc.sync.dma_start(rq_sb[:, 0:1024], rq[:, ds(0, 1024)])
        nc.sync.dma_start(rq_sb[:, 2048:3072], rq[:, ds(2048, 1024)])
        nc.sync.dma_start(tri_sb[:], tri)
        nc.sync.dma_start(rq_sb[0:64, 5120:6144], rq[0:64, ds(5120, 1024)])
        nc.sync.dma_start(rq_sb[0:64, 7168:8192], rq[0:64, ds(7168, 1024)])
        nc.sync.dma_start(rq_sb[:, 1024:2048], rq[:, ds(1024, 1024)])
        nc.sync.dma_start(rq_sb[:, 3072:4096], rq[:, ds(3072, 1024)])
        nc.sync.dma_start(wo_sb[:], wo)
        if FP8C >= 0:
            nc.sync.dma_start(wo8a_sb[:].rearrange("p a b -> p (a b)"), wo8a)
            nc.sync.dma_start(wo8b_sb[:].rearrange("p a b -> p (a b)"), wo8b)
        # ones column for the softmax denominator (tri[p,127] == 1 for all p)
        nc.gpsimd.tensor_copy(
            v65[:, :, 64:65],
            tri_sb[:, 127:128][:, None, :].to_broadcast([128, 16, 1]),
        )

        # ---------------- work-unit machinery ----------------
        pump_q = []

        def pump(k):
            for _ in range(min(int(k), len(pump_q))):
                pump_q.pop(0)()

        def drain():
            while pump_q:
                pump_q.pop(0)()

        # ---------------- projection ----------------
        RAWS = (q01r, q23r, kvr)

        def proj_term(acc, m, t, n2, hf, start, stop):
            for ti, (wt, ht) in enumerate(((wqa_sb, ha_sb), (wqb_sb, ha_sb), (wqa_sb, hb_sb))):
                nc.tensor.matmul(
                    acc[:],
                    wt[:, m, 2 * t : 2 * t + 2, :],
                    ht[:, 2 * t : 2 * t + 2, ds(1024 * hf + 512 * n2, 512)],
                    start=(start and ti == 0),
                    stop=(stop and ti == 2),
                    perf_mode=DR,
                )

        def proj_units(hf, m):
            u = []
            for n2 in range(2):
                slot = {}
                for t in range(8):
                    def mm(t=t, n2=n2, slot=slot):
                        if t == 0:
                            slot["acc"] = psA.tile(
                                [128, 512], FP, tag="acc", bufs=2,
                                name=f"acc_{hf}_{m}_{n2}",
                            )
                        proj_term(slot["acc"], m, t, n2, hf, t == 0, t == 7)
                    u.append(mm)
                def cp(slot=slot, n2=n2, m=m):
                    nc.scalar.copy(RAWS[m][:, ds(1024 * hf + 512 * n2, 512)], slot["acc"][:])
                u.append(cp)
            return u

        # ---------------- rope ----------------
        SWAP_MASK = [i ^ 1 for i in range(32)]

        def rope_half(dst, raw, p, tb, hf, swname, base=0, w=1024):
            cs = ds(1024 * hf + base, w)
            cosap = rq_sb[0:p, tb + 1024 * hf + base : tb + 1024 * hf + base + w]
            sinap = rq_sb[0:p, tb + 2048 + 1024 * hf + base : tb + 2048 + 1024 * hf + base + w]
            sw = scr.tile([128, 1024], BF, tag="sc", name=f"sw_{swname}{hf}")
            nc.vector.stream_shuffle(sw[0:p, 0:w], raw[0:p, cs], SWAP_MASK)
            t0 = scr.tile([128, 1024], BF, tag="sc", name=f"t0_{swname}{hf}")
            nc.vector.tensor_mul(t0[0:p, 0:w], raw[0:p, cs], cosap)
            nc.vector.tensor_mul(raw[0:p, cs], sw[0:p, 0:w], sinap)
            nc.vector.tensor_add(dst[0:p, cs], t0[0:p, 0:w], raw[0:p, cs])

        def krope_units(hf, pool, ptag, pbufs):
            if hf == 0:
                # first scores wait on k-rope + dup: pipeline 512-chunks
                u = []
                for base in (0, 512):
                    u.append(lambda base=base: rope_half(ks, kvr, 64, 4096, hf, "k", base, 512))
                    u.append(lambda base=base: nc.sync.dma_start(
                        ks[64:128, ds(1024 * hf + base, 512)],
                        ks[0:64, ds(1024 * hf + base, 512)]))
            else:
                u = [lambda: rope_half(ks, kvr, 64, 4096, hf, "k")]
                u.append(lambda: nc.sync.dma_start(
                    ks[64:128, ds(1024 * hf, 1024)], ks[0:64, ds(1024 * hf, 1024)]))
            for b in (2 * hf, 2 * hf + 1):
                def vt_b(b=b):
                    vt = pool.tile([128, 256], BF, tag=ptag, bufs=pbufs, name=f"vt_{b}")
                    for q in range(4):
                        j = 4 * b + q
                        nc.tensor.transpose(
                            vt[:, ds(64 * q, 64)],
                            kvr[64:128, ds(128 * j, 128)],
                            ident64,
                        )
                    nc.vector.tensor_copy(
                        v65[:, 4 * b : 4 * b + 4, 0:64],
                        vt[:].rearrange("p (q c) -> p q c", c=64),
                    )
                u.append(vt_b)
            return u

        # ---------------- attention ----------------
        pending_norm = [None]
        pending_opq = []

        def emit_norm_pre(c, hp, po):
            poc = pocp.tile([65, 1024], BF, tag="poc", name=f"poc_{c}_{hp}")
            if c == 3 and hp == 1:
                # the final norm gates the tail; Act is idle once exps finish
                nc.scalar.copy(poc[:, :], po[0:65, :])
            else:
                nc.vector.tensor_copy(poc[:, :], po[0:65, :])
            return poc

        def emit_norm_post(c, hp, po, poc):
            # hh=1 (odd head) first: its o2 half needs an extra partition-move
            # DMA that outproj waits on, so start that chain earliest
            dsttile = o2a if hp == 0 else o2b
            rbr = rbrp.tile([64, 1024], BF, tag="rbr", name=f"rbr_{c}_{hp}")
            for half in (1, 0):
                cs = ds(512 * half, 512)
                nc.tensor.matmul(
                    po[0:64, cs],
                    ones_row,
                    poc[64:65, cs],
                    start=True,
                    stop=True,
                    skip_group_check=True,
                )
                nc.vector.reciprocal(rbr[0:64, cs], po[0:64, cs])
                if half == 1:
                    nm = nmp.tile([64, 512], BF, tag="nm", name=f"nm_{c}_{hp}")
                    nc.gpsimd.tensor_mul(nm[0:64, :], poc[0:64, cs], rbr[0:64, cs])
                    nc.sync.dma_start(dsttile[64:128, ds(512 * c, 512)], nm[0:64, :])
                else:
                    nc.gpsimd.tensor_mul(
                        dsttile[0:64, ds(512 * c, 512)], poc[0:64, cs], rbr[0:64, cs]
                    )
            if c <= FP8C:
                # split this head-pair's o2 half as soon as it is complete
                cs = ds(512 * c, 512)
                src = o2a if hp == 0 else o2b
                nc.gpsimd.tensor_copy(o28h[:, hp, cs], src[:, cs])
                nc.gpsimd.tensor_sub(o28l[:, hp, cs], src[:, cs], o28h[:, hp, cs])
            if hp == 1:
                pending_opq.append(outproj_units(c))

        def attention(c, hp, pump_rate):
            nj = 4 * c + 4
            if pending_norm[0] is not None:
                pc, php, ppo = pending_norm[0]
                ppoc = emit_norm_pre(pc, php, ppo)
                pending_norm[0] = (pc, php, ppo, ppoc)
            po = P["so"].tile([128, 1024], FP, tag="po", bufs=1, name=f"po_{c}_{hp}")
            exs = {}
            offs = {}
            for j in range(nj):
                r = j - 4 * c
                off = 128 * r if r >= 0 else 0
                offs[j] = off
                if pump_rate == 0.5:
                    pump(1 if j % 2 == 1 else 0)
                else:
                    pump(pump_rate)
                ps = P["so"].tile([128, 1024], FP, tag="sc", bufs=2, name=f"ps_{c}_{hp}_{j}")
                qt = q01s if hp == 0 else q23s
                for hh in range(2):
                    base = 64 * hh
                    nc.tensor.matmul(
                        ps[:, ds(512 * hh + off, 512 - off)],
                        ks[base : base + 64, ds(128 * j, 128)],
                        qt[base : base + 64, ds(512 * c + off, 512 - off)],
                        skip_group_check=True,
                    )
                ex = expool.tile([128, 1024], BF, tag="ex", name=f"ex_{c}_{hp}_{j}")
                exs[j] = ex
                psv = ps.rearrange("p (h w) -> p h w", w=512)[:, :, ds(off, 512 - off)]
                exv = ex.rearrange("p (h w) -> p h w", w=512)[:, :, ds(off, 512 - off)]
                nc.scalar.activation(exv, psv, EXP)
                if r >= 0:
                    exd = ex.rearrange("p (h w) -> p h w", w=512)[:, :, ds(off, 128)]
                    eng = nc.vector
                    eng.tensor_mul(
                        exd,
                        exd,
                        tri_sb[:, 0:128][:, None, :].to_broadcast([128, 2, 128]),
                    )
                if j == 0 and pending_norm[0] is not None:
                    emit_norm_post(*pending_norm[0])
                    pending_norm[0] = None
                if j == 3 and pending_opq:
                    pump_q.extend(pending_opq.pop(0))
                if j > 0:
                    pv(po, exs[j - 1], j - 1, nj, offs[j - 1], c)
            pump(pump_rate)
            pv(po, exs[nj - 1], nj - 1, nj, offs[nj - 1], c)
            pending_norm[0] = (c, hp, po)

        def pv(po, ex, j, nj, off, c):
            for hh in range(2):
                nc.tensor.matmul(
                    po[0:65, ds(512 * hh + off, 512 - off)],
                    v65[:, j, :],
                    ex[:, ds(512 * hh + off, 512 - off)],
                    start=(j == 0),
                    stop=(j == nj - 1),
                    skip_group_check=True,
                )

        # ---------------- out-projection ----------------
        def outproj_units(c):
            # fine-grained units (one [128,512] psum tile each) so pumping one
            # into an attention j-loop never starves the Act-paced exp chain
            u = []
            sts = {}
            for b in range(4):
                for n in range(4):
                    def one(b=b, n=n):
                        if n == 0:
                            sts[b] = stp.tile(
                                [128, 2048], BF, tag="st", name=f"st_{c}_{b}"
                            )
                        st = sts[b]
                        pp = psA.tile([128, 512], FP, tag="acc", bufs=2, name=f"pp_{c}_{b}_{n}")
                        if c <= FP8C:
                            bs = ds(512 * c + 128 * b, 128)
                            ns = ds(512 * n, 512)
                            terms = (
                                (o28h, wo8a_sb),
                                (o28l, wo8a_sb),
                                (o28h, wo8b_sb),
                            )
                            for ti, (ot, wt) in enumerate(terms):
                                nc.tensor.matmul(
                                    pp[:],
                                    ot[:, :, bs],
                                    wt[:, :, ns],
                                    start=(ti == 0),
                                    stop=(ti == 2),
                                    perf_mode=DR,
                                )
                        else:
                            nc.tensor.matmul(
                                pp[:],
                                o2a[:, ds(512 * c + 128 * b, 128)],
                                wo_sb[:, ds(512 * n, 512)],
                                start=True,
                                stop=False,
                            )
                            nc.tensor.matmul(
                                pp[:],
                                o2b[:, ds(512 * c + 128 * b, 128)],
                                wo_sb[:, ds(2048 + 512 * n, 512)],
                                start=False,
                                stop=True,
                            )
                        on_act = False
                        if on_act:
                            nc.scalar.copy(st[:, ds(512 * n, 512)], pp[:])
                        else:
                            nc.vector.tensor_copy(st[:, ds(512 * n, 512)], pp[:])
                        if n == 3:
                            nc.sync.dma_start(out[ds(512 * c + 128 * b, 128), :], st[:])
                    u.append(one)
            return u

        # ---------------- schedule ----------------
        # hf=0 projection: all three m-chunks accumulate in 6 transient PSUM
        # banks, interleaved per arriving h g-pair (m0/m1 lag one t so their
        # first matmuls don't stall on the later wq DMAs); projection then
        # completes right after the last h byte lands.
        accs6 = {}
        for m in (2, 0, 1):
            for n2 in range(2):
                accs6[(m, n2)] = psP6.tile(
                    [128, 512], FP, tag="p6", bufs=6, name=f"acc6_{m}_{n2}"
                )
        for t in range(10):
            if t < 8:
                for n2 in range(2):
                    proj_term(accs6[(2, n2)], 2, t, n2, 0, t == 0, t == 7)
            if 1 <= t <= 8:
                for n2 in range(2):
                    proj_term(accs6[(0, n2)], 0, t - 1, n2, 0, t == 1, t == 8)
            if t >= 2:
                for n2 in range(2):
                    proj_term(accs6[(1, n2)], 1, t - 2, n2, 0, t == 2, t == 9)
        for m in (2, 0, 1):
            for n2 in range(2):
                nc.scalar.copy(RAWS[m][:, ds(512 * n2, 512)], accs6[(m, n2)][:])
            if m == 2:
                for unit in krope_units(0, psP6, "p6", 6):
                    unit()
            elif m == 0:
                # chunk c only reads q cols [512c:512c+512]: emit the first
                # 512 immediately so attention(0,0) ungates sooner
                rope_half(q01s, q01r, 128, 0, 0, "q01", 0, 512)
                rope_half(q01s, q01r, 128, 0, 0, "q01", 512, 512)
            else:
                rope_half(q23s, q23r, 128, 0, 0, "q23", 0, 512)
                rope_half(q23s, q23r, 128, 0, 0, "q23", 512, 512)
        p6ctx.close()
        soctx = ExitStack()
        P["so"] = soctx.enter_context(tc.tile_pool(name="psSO", bufs=1, space="PSUM"))

        pump_q.extend(proj_units(1, 2))
        pump_q.extend(krope_units(1, psA, "acc", 2))
        pump_q.extend(proj_units(1, 0))
        pump_q.append(lambda: rope_half(q01s, q01r, 128, 0, 1, "q01"))
        pump_q.extend(proj_units(1, 1))
        pump_q.append(lambda: rope_half(q23s, q23r, 128, 0, 1, "q23"))

        pump(3)
        attention(0, 0, 3)
        attention(0, 1, 3)
        for hp in range(2):
            attention(1, hp, 2)
        drain()

        # attention c=2,3; outproj(c) units are queued by emit_norm_post
        for hp in range(2):
            attention(2, hp, 0.5)
        for hp in range(2):
            attention(3, hp, 0.5)
        pc, php, ppo = pending_norm[0]
        ppoc = emit_norm_pre(pc, php, ppo)
        emit_norm_post(pc, php, ppo, ppoc)
        pending_norm[0] = None
        pending_opq.clear()  # outproj(3) handled by the pipelined tail below
        drain()
        soctx.close()
        psT = ctx.enter_context(tc.tile_pool(name="psT", bufs=1, space="PSUM"))
        DEPTH = 6
        pps = {}
        sts = {}

        def tail_a(idx):
            b, n = divmod(idx, 4)
            pp = psT.tile([128, 512], FP, tag="pt", bufs=6, name=f"tpp_{idx}")
            pps[idx] = pp
            nc.tensor.matmul(
                pp[:],
                o2a[:, ds(512 * 3 + 128 * b, 128)],
                wo_sb[:, ds(512 * n, 512)],
                start=True,
                stop=False,
            )

        for idx in range(DEPTH):
            tail_a(idx)
        for idx in range(16):
            b, n = divmod(idx, 4)
            if n == 0:
                sts[b] = stp.tile([128, 2048], BF, tag="st", name=f"tst_{b}")
            nc.tensor.matmul(
                pps[idx][:],
                o2b[:, ds(512 * 3 + 128 * b, 128)],
                wo_sb[:, ds(2048 + 512 * n, 512)],
                start=False,
                stop=True,
            )
            if n % 2 == 0:
                nc.vector.tensor_copy(sts[b][:, ds(512 * n, 512)], pps[idx][:])
            else:
                nc.scalar.copy(sts[b][:, ds(512 * n, 512)], pps[idx][:])
            if idx + DEPTH < 16:
                tail_a(idx + DEPTH)
            if n == 1:
                nc.sync.dma_start(
                    out[ds(512 * 3 + 128 * b, 128), ds(0, 1024)], sts[b][:, 0:1024]
                )
            elif n == 3:
                nc.sync.dma_start(
                    out[ds(512 * 3 + 128 * b, 128), ds(1024, 1024)], sts[b][:, 1024:2048]
                )

    nc.compile()
    return nc


def get_module():
    if "nc" not in _CACHE:
        _CACHE["nc"] = _build_module()
    return _CACHE["nc"]


def _pack16(x):
    # [16*128, N] -> [128, 16*N] with [p, N*g + n] = x[128*g + p, n]
    n = x.shape[1]
    return (
        np.ascontiguousarray(x.reshape(16, 128, n).transpose(1, 0, 2)).reshape(128, 16 * n)
    )


def _split8(x):
    import ml_dtypes

    hi = x.astype(ml_dtypes.float8_e4m3fn)
    lo = (x - hi.astype(np.float32)).astype(ml_dtypes.float8_e4m3fn)
    return hi.view(np.uint8), lo.view(np.uint8)


def _bf(x):
    import ml_dtypes

    return x.astype(ml_dtypes.bfloat16).view(np.uint16)


def prep_inputs(hidden_states, freqs_cis, wqkv, wo):
    h = np.asarray(hidden_states, dtype=np.float32)[0]  # [S, D]
    fc = np.asarray(freqs_cis, dtype=np.float32)  # [S, 32, 2]
    wqkv = np.asarray(wqkv, dtype=np.float32)  # [3072, D]
    wo = np.asarray(wo, dtype=np.float32)  # [D, D]

    hT_sb = _pack16(np.ascontiguousarray(h.T))  # [128, 16*2048]
    h8a, h8b = _split8(hT_sb)

    cos = fc[:, :, 0]  # [S, 32]
    sin = fc[:, :, 1]
    cos_ext = np.repeat(cos, 2, axis=1).T  # [64, S]
    sgn = np.where(np.arange(HD) % 2 == 0, -1.0, 1.0).astype(np.float32)[:, None]
    sin_ext = np.repeat(sin, 2, axis=1).T * sgn
    scale = 1.0 / (np.sqrt(np.float32(HD)) * WSCALE)
    kscale = 1.0 / WSCALE
    rq_np = np.concatenate(
        [
            np.tile(cos_ext * scale, (2, 1)),
            np.tile(sin_ext * scale, (2, 1)),
            np.tile(cos_ext * kscale, (2, 1)),
            np.tile(sin_ext * kscale, (2, 1)),
        ],
        axis=1,
    ).astype(np.float32)  # [128, 8192]
    tri_np = np.concatenate(
        [
            (np.arange(128)[:, None] <= np.arange(128)[None, :]).astype(np.float32),
            np.eye(128, dtype=np.float32),
        ],
        axis=1,
    )  # [128, 256]

    in_maps = []
    for i in range(NCORES):
        wl = np.concatenate(
            [
                wqkv[256 * i : 256 * i + 256],
                wqkv[D + 64 * i : D + 64 * i + 64],
                wqkv[D + KV_SIZE + 64 * i : D + KV_SIZE + 64 * i + 64],
            ],
            axis=0,
        )  # [384, D]
        wq_sb = _pack16(np.ascontiguousarray(wl.T)) * WSCALE  # [128, 16*384]
        # g-major [128, 16, 3, 128] -> m-major [128, 3, 16, 128]
        wq_sb = np.ascontiguousarray(
            wq_sb.reshape(128, 16, 3, 128).transpose(0, 2, 1, 3)
        ).reshape(128, 16 * 384)
        wqa, wqb = _split8(wq_sb)
        woT = np.ascontiguousarray(wo[:, 256 * i : 256 * i + 256].T)  # [256, D]
        wo_pack = np.ascontiguousarray(
            woT.reshape(2, 128, D).transpose(1, 0, 2)
        ).reshape(128, 2 * D)
        wo_sb = wo_pack * np.float32(1.0 / WSCALE)
        wo8a_np, wo8b_np = _split8(wo_pack * np.float32(WSCALE))
        in_maps.append(
            {
                "h8a": h8a,
                "h8b": h8b,
                "wqa": wqa,
                "wqb": wqb,
                "wo": _bf(wo_sb),
                "wo8a": wo8a_np,
                "wo8b": wo8b_np,
                "rq": _bf(rq_np),
                "tri": _bf(tri_np),
            }
        )
    return in_maps


def run_on_hw(in_maps, trace=False, **kw):
    from concourse.bass_utils import run_bass_kernel_spmd

    nc = get_module()
    return run_bass_kernel_spmd(nc, in_maps, list(range(NCORES)), trace=trace, **kw)


def kernel(hidden_states, freqs_cis, wqkv, wo):
    import ml_dtypes

    in_maps = prep_inputs(hidden_states, freqs_cis, wqkv, wo)
    res = run_on_hw(in_maps)
    acc = np.zeros((S, D), dtype=np.float32)
    for r in res.results:
        o = r["out"]
        if o.dtype == np.uint16:
            o = o.view(ml_dtypes.bfloat16)
        acc += o.astype(np.float32)
    if FP8C >= 0:
        acc[0 : (FP8C + 1) * 512] *= np.float32(2.0**-16)  # fp8 outproj rows carry x65536
    return acc.reshape(1, S, D)
